# revision 3
# baseline (speedup 1.0000x reference)
"""KNN kernel v16 for Trainium2 (8 NeuronCores, SPMD).

Problem: query [2, 8192, 256] f32, support [2, 16384, 256] f32.
Returns (values [2, 8192, 16] f32 ascending Euclidean distances,
         idx    [2, 8192, 16] int32).

Compute design (from v15, ~1.2ms/core simulated):
- Matmul: 4-product bf16 split precision.  q = q_hi + q_lo, s = s_hi + s_lo
  (bf16 splits, host-computed); score = q_hi.s_hi + q_hi.s_lo + q_lo.s_hi +
  q_lo.s_lo.  Keeping the lo.lo term (vs v15) drops the score noise from
  ~7e-5 to ~1e-7, making top-k tie mismatches vs the f32 reference rare.
- The -0.5*|s|^2 column offset is seeded into PSUM with one ACT copy from a
  resident [128, N] replica; matmuls accumulate on top (start=False).
- Selection on 1024-wide spans: one [128,1024] 2-bank PSUM tile per
  (span, m-tile), one ACT evict, one DVE max8 + max_index -> 8 candidates
  per span, 128 candidates per query row.
- Index recovery per m-tile: one-hot on u16 (4x DVE perf mode) + ACT
  accum reduce, on the [128,128] candidate array.
- m-blocking: support re-streamed per block; each block's finalize/recovery
  DVE work is emitted interleaved into the next block's main loop so it
  overlaps PE instead of serializing into a tail.
- Single f32 output [m_core, 32]: cols 0:16 = sorted distances, cols 16:32 =
  neighbor indices as exact f32 integers (one output tensor -> one fetch).

Dispatch design (the dominant cost in this axon-tunneled container):
- The host<->device tunnel moves ~46 MB/s with ~80 ms/RPC latency, while
  device<->device copies on the terminal run at ~1.7 GB/s.  So kernel()
  stages inputs into device HBM once per call -- uploading only unique
  bytes (query shards per core; support/s2 once per batch, replicated to
  sibling cores d2d) -- mirroring the native run_neff contract where
  ExternalInputs are pre-written to HBM before execution.
- The NEFF execution itself is dispatched through a module-cached
  jit(shard_map(bass_exec)) (same lowering run_bass_kernel_spmd uses under
  axon), so repeat dispatches skip retracing and re-upload.
"""

import time

import numpy as np

import concourse.bacc as bacc
import concourse.mybir as mybir
import concourse.tile as tile
from concourse.alu_op_type import AluOpType

dt = mybir.dt

B = 2
M = 8192
N = 16384
C = 256
K = 16
NCORES = 8
M_CORE = B * M // NCORES  # 2048

NEG_BIG = -3.0e38
SELW = 1024  # selection span (2 PSUM banks)


def build_knn_kernel(m_core=M_CORE, n=N, c=C, blocks=(2, 2, 2, 2, 2, 2, 2, 2)):
    nsel = n // SELW  # spans per row
    ncand = 8 * nsel  # candidates per query row (128)
    mt = m_core // 128
    assert sum(blocks) == mt and c == 256

    nc = bacc.Bacc(None, target_bir_lowering=False)
    qh_d = nc.dram_tensor("q_hi", [c, m_core], dt.bfloat16, kind="ExternalInput")
    ql_d = nc.dram_tensor("q_lo", [c, m_core], dt.bfloat16, kind="ExternalInput")
    sh_d = nc.dram_tensor("s_hi", [c, n], dt.bfloat16, kind="ExternalInput")
    sl_d = nc.dram_tensor("s_lo", [c, n], dt.bfloat16, kind="ExternalInput")
    s2_d = nc.dram_tensor("s2rep", [128, n], dt.float32, kind="ExternalInput")
    q2_d = nc.dram_tensor("q2", [128, mt], dt.float32, kind="ExternalInput")
    q2h_d = nc.dram_tensor("q2h", [128, mt], dt.float32, kind="ExternalInput")
    cbase_d = nc.dram_tensor("cbase", [128, ncand], dt.uint16, kind="ExternalInput")
    iota_d = nc.dram_tensor("iotac", [128, ncand], dt.uint16, kind="ExternalInput")
    out_d = nc.dram_tensor("out", [m_core, 2 * K], dt.float32, kind="ExternalOutput")

    with tile.TileContext(nc) as tc:
        with (
            tc.tile_pool(name="persist", bufs=1) as persist,
            tc.tile_pool(name="stream", bufs=3) as stream,
            tc.tile_pool(name="selp", bufs=4) as selp,
            tc.tile_pool(name="fin", bufs=2) as fin,
            tc.tile_pool(name="ps", bufs=2, space="PSUM") as ps,
        ):
            # ---------------- preamble: resident tensors ----------------
            s2rep_t = persist.tile([128, n], dt.float32, tag="s2rep")
            nchk = max(n // 8, 2 * SELW)
            qh_t = persist.tile([128, 2 * m_core], dt.bfloat16, tag="qh")
            ql_t = persist.tile([128, 2 * m_core], dt.bfloat16, tag="ql")
            nc.scalar.dma_start(out=s2rep_t[:, 0:nchk], in_=s2_d[:, 0:nchk])
            nc.scalar.dma_start(out=qh_t[:, 0:m_core], in_=qh_d[0:128, :])
            nc.scalar.dma_start(
                out=qh_t[:, m_core : 2 * m_core], in_=qh_d[128:256, :]
            )
            for cc in range(2):
                nc.scalar.dma_start(
                    out=ql_t[:, cc * m_core : (cc + 1) * m_core],
                    in_=ql_d[cc * 128 : (cc + 1) * 128, :],
                )
            q2_t = persist.tile([128, mt], dt.float32, tag="q2")
            q2h_t = persist.tile([128, mt], dt.float32, tag="q2h")
            cbase_t = persist.tile([128, ncand], dt.uint16, tag="cbase")
            iota_t = persist.tile([128, ncand], dt.uint16, tag="iota")

            late = []  # deferred preamble DMAs, drained into early loop slots
            for cc in range(1, n // nchk):
                late.append((lambda cc=cc: nc.sync.dma_start(
                    out=s2rep_t[:, cc * nchk : (cc + 1) * nchk],
                    in_=s2_d[:, cc * nchk : (cc + 1) * nchk],
                )))
            late.append(lambda: nc.sync.dma_start(out=q2_t, in_=q2_d[:, :]))
            late.append(lambda: nc.sync.dma_start(out=q2h_t, in_=q2h_d[:, :]))
            late.append(lambda: nc.sync.dma_start(out=cbase_t, in_=cbase_d[:, :]))
            late.append(lambda: nc.sync.dma_start(out=iota_t, in_=iota_d[:, :]))

            # candidate values / local positions, all m-tiles
            cval_t = persist.tile([128, mt * ncand], dt.float32, tag="cval")
            cpos_t = persist.tile([128, mt * ncand], dt.uint16, tag="cpos")

            # ------------- finalize: emitted as micro-op closures -------------
            def finalize_steps(m, last=False):
                """Return a list of closures; each emits one small op-group.
                Calling them spread across later main-loop iterations lets the
                finalize DVE work interleave with selection instead of
                bursting."""
                cv = cval_t[:, m * ncand : (m + 1) * ncand]
                cp = cpos_t[:, m * ncand : (m + 1) * ncand]
                st = {}

                def s0():
                    st["cg"] = fin.tile([128, ncand], dt.uint16, tag="cg", name="cg")
                    nc.vector.tensor_tensor(
                        out=st["cg"], in0=cp, in1=cbase_t, op=AluOpType.add
                    )
                    st["val16"] = fin.tile([128, K], dt.float32, tag="val16", name="val16")
                    st["pos16"] = fin.tile([128, K], dt.uint16, tag="pos16", name="pos16")
                    nc.vector.max(out=st["val16"][:, 0:8], in_=cv)
                    nc.vector.max_index(
                        out=st["pos16"][:, 0:8], in_max=st["val16"][:, 0:8], in_values=cv
                    )

                def s1():
                    st["cv2"] = fin.tile([128, ncand], dt.float32, tag="cv2", name="cv2")
                    nc.vector.match_replace(
                        out=st["cv2"],
                        in_to_replace=st["val16"][:, 0:8],
                        in_values=cv,
                        imm_value=NEG_BIG,
                    )
                    nc.vector.max(out=st["val16"][:, 8:16], in_=st["cv2"])
                    nc.vector.max_index(
                        out=st["pos16"][:, 8:16],
                        in_max=st["val16"][:, 8:16],
                        in_values=st["cv2"],
                    )

                def s2():
                    st["idxf"] = fin.tile([128, K], dt.float32, tag="idxf", name="idxf")
                    st["scr"] = fin.tile([128, ncand], dt.uint16, tag="scr", name="scr")

                def mk_rec(r):
                    def rec():
                        msk = fin.tile(
                            [128, ncand], dt.uint16, tag=f"msk{r % 4}", name=f"msk{r % 4}"
                        )
                        nc.vector.scalar_tensor_tensor(
                            out=msk,
                            in0=iota_t,
                            scalar=st["pos16"][:, r : r + 1],
                            in1=st["cg"],
                            op0=AluOpType.is_equal,
                            op1=AluOpType.mult,
                        )
                        if last and r % 2 == 0:
                            # final block: split reduces DVE/ACT to shorten the tail
                            with nc.allow_low_precision(reason="u16 one-hot sum exact"):
                                nc.vector.tensor_reduce(
                                    out=st["idxf"][:, r : r + 1],
                                    in_=msk,
                                    axis=mybir.AxisListType.X,
                                    op=AluOpType.add,
                                )
                        else:
                            # free-dim sum via ACT accum_out (offloads DVE)
                            nc.scalar.activation(
                                out=st["scr"],
                                in_=msk,
                                func=mybir.ActivationFunctionType.Copy,
                                accum_out=st["idxf"][:, r : r + 1],
                            )
                    return rec

                def s3():
                    # indices, as exact f32 integers, into cols 16:32
                    nc.sync.dma_start(
                        out=out_d[m * 128 : (m + 1) * 128, K : 2 * K], in_=st["idxf"]
                    )
                    vc = fin.tile([128, K], dt.float32, tag="vc", name="vc")
                    nc.vector.tensor_scalar(
                        out=vc,
                        in0=st["val16"],
                        scalar1=q2h_t[:, m : m + 1],
                        scalar2=None,
                        op0=AluOpType.min,
                    )
                    dist = fin.tile([128, K], dt.float32, tag="dist", name="dist")
                    nc.scalar.activation(
                        out=dist,
                        in_=vc,
                        func=mybir.ActivationFunctionType.Sqrt,
                        bias=q2_t[:, m : m + 1],
                        scale=-2.0,
                    )
                    nc.sync.dma_start(
                        out=out_d[m * 128 : (m + 1) * 128, 0:K], in_=dist
                    )

                return [s0, s1, s2] + [mk_rec(r) for r in range(K)] + [s3]

            pending = []  # finalize micro-ops awaiting emission
            drain_acc = [0.0]

            def drain_some(k):
                drain_acc[0] += k
                while drain_acc[0] >= 1.0:
                    drain_acc[0] -= 1.0
                    if pending:
                        pending.pop(0)()

            # ---------------- main loop: m-blocks, spans, m-tiles ----------
            m_start = 0
            for blk_i, blk in enumerate(blocks):
                m_list = list(range(m_start, m_start + blk))
                m_start += blk
                PW = 2 * SELW  # span-pair width (4 PSUM banks)
                for tp in range(nsel // 2):
                    sh_t = stream.tile([128, 2 * PW], dt.bfloat16, tag="sh")
                    sl_t = stream.tile([128, 2 * PW], dt.bfloat16, tag="sl")
                    for d_, t_ in ((sh_d, sh_t), (sl_d, sl_t)):
                        for cc in range(2):
                            nc.sync.dma_start(
                                out=t_[:, cc * PW : (cc + 1) * PW],
                                in_=d_[cc * 128 : (cc + 1) * 128, tp * PW : (tp + 1) * PW],
                            )

                    if late:
                        late.pop(0)()
                    for m in m_list:
                        psum = ps.tile([128, PW], dt.float32, tag="p")
                        nc.scalar.copy(
                            out=psum, in_=s2rep_t[:, tp * PW : (tp + 1) * PW]
                        )
                        for h in range(4):
                            pv = psum[:, h * 512 : (h + 1) * 512]
                            for pi, (lhsT, rhsT) in enumerate((
                                (qh_t, sh_t), (qh_t, sl_t), (ql_t, sh_t), (ql_t, sl_t)
                            )):
                                for cc in range(2):
                                    nc.tensor.matmul(
                                        pv,
                                        lhsT=lhsT[:, cc * m_core + m * 128 : cc * m_core + (m + 1) * 128],
                                        rhs=rhsT[:, cc * PW + h * 512 : cc * PW + (h + 1) * 512],
                                        start=False,
                                        stop=(pi == 3 and cc == 1),
                                        skip_group_check=True,
                                    )
                        sc = selp.tile([128, PW], dt.float32, tag="sc", name="sc")
                        nc.scalar.copy(out=sc, in_=psum)
                        for hs in range(2):
                            jp2 = 2 * tp + hs
                            scv = sc[:, hs * SELW : (hs + 1) * SELW]
                            cv8 = cval_t[:, m * ncand + jp2 * 8 : m * ncand + (jp2 + 1) * 8]
                            nc.vector.max(out=cv8, in_=scv)
                            nc.vector.max_index(
                                out=cpos_t[:, m * ncand + jp2 * 8 : m * ncand + (jp2 + 1) * 8],
                                in_max=cv8,
                                in_values=scv,
                            )
                        drain_some(2)
                    if blk_i == 0 and tp == nsel // 2 - 1:
                        while late:
                            late.pop(0)()
                    if tp == nsel // 2 - 1:
                        is_last = blk_i == len(blocks) - 1
                        for m in m_list:
                            pending.extend(finalize_steps(m, last=is_last))
            # drain any remaining finalize work
            drain_some(len(pending))

    nc.finalize()
    return nc


_NC_CACHE = {}


def _get_nc():
    key = (M_CORE, N, C)
    if key not in _NC_CACHE:
        _NC_CACHE[key] = build_knn_kernel()
    return _NC_CACHE[key]


def _bf16_split(x, parts):
    """Split fp32 array into `parts` bf16 arrays summing to ~x."""
    import ml_dtypes

    out = []
    resid = x.astype(np.float32)
    for _ in range(parts):
        p = resid.astype(ml_dtypes.bfloat16)
        out.append(p)
        resid = resid - p.astype(np.float32)
    return out


def _prep_in_maps(query, support):
    rows = M // (NCORES // B)  # 2048
    mt = M_CORE // 128
    nsel = N // SELW
    ncand = 8 * nsel
    cb = (np.arange(nsel, dtype=np.uint16) * SELW).repeat(8)
    cbase = np.broadcast_to(cb, (128, ncand)).copy()
    iotac = np.broadcast_to(np.arange(ncand, dtype=np.uint16), (128, ncand)).copy()
    per_batch = {}
    for b in range(B):
        s = support[b]
        sh, sl = _bf16_split(np.ascontiguousarray(s.T), 2)
        s2 = (-0.5 * (s.astype(np.float64) ** 2).sum(1)).astype(np.float32)
        per_batch[b] = (sh, sl, np.broadcast_to(s2, (128, s2.shape[0])).copy())

    in_maps = []
    for core in range(NCORES):
        b = core // (NCORES // B)
        r0 = (core % (NCORES // B)) * rows
        qs = query[b, r0 : r0 + rows]  # [2048, 256]
        qh, ql = _bf16_split(np.ascontiguousarray(qs.T), 2)
        q2 = (qs.astype(np.float64) ** 2).sum(1).astype(np.float32)
        sh, sl, s2rep = per_batch[b]
        in_maps.append(
            {
                "q_hi": qh,
                "q_lo": ql,
                "s_hi": sh,
                "s_lo": sl,
                "s2rep": s2rep,
                "q2": np.ascontiguousarray(q2.reshape(mt, 128).T),
                "q2h": np.ascontiguousarray((q2 * 0.5).reshape(mt, 128).T),
                "cbase": cbase,
                "iotac": iotac,
            }
        )
    return in_maps


# --------------------------------------------------------------------------
# Dispatch layer: cached jit(shard_map(bass_exec)) + HBM input staging.
# --------------------------------------------------------------------------

# Tensors whose value is shared by groups of cores: replicated terminal-side
# via device-to-device copies instead of re-uploading through the tunnel.
_REPLICATED = {"s_hi": 4, "s_lo": 4, "s2rep": 4, "cbase": 8, "iotac": 8}

_DISPATCH_CACHE = {}


def _get_dispatch():
    if "d" in _DISPATCH_CACHE:
        return _DISPATCH_CACHE["d"]
    import jax
    from jax.sharding import Mesh, PartitionSpec, NamedSharding
    from jax.experimental.shard_map import shard_map
    from concourse.bass2jax import (
        _bass_exec_p,
        install_neuronx_cc_hook,
        partition_id_tensor,
    )

    nc = _get_nc()
    install_neuronx_cc_hook()
    partition_name = (
        nc.partition_id_tensor.name if nc.partition_id_tensor else None
    )
    in_names, out_names, out_avals, zero_shapes = [], [], [], []
    for alloc in nc.m.functions[0].allocations:
        if not isinstance(alloc, mybir.MemoryLocationSet):
            continue
        name = alloc.memorylocations[0].name
        if alloc.kind == "ExternalInput":
            if name != partition_name:
                in_names.append(name)
        elif alloc.kind == "ExternalOutput":
            out_names.append(name)
            shape = tuple(alloc.tensor_shape)
            np_dt = mybir.dt.np(alloc.dtype)
            out_avals.append(jax.core.ShapedArray(shape, np_dt))
            zero_shapes.append((shape, np_dt))
    n_params = len(in_names)
    n_outs = len(out_avals)
    in_names_all = in_names + out_names + (
        [partition_name] if partition_name else []
    )

    def _body(*args):
        operands = list(args)
        if partition_name is not None:
            operands.append(partition_id_tensor())
        outs = _bass_exec_p.bind(
            *operands,
            out_avals=tuple(out_avals),
            in_names=tuple(in_names_all),
            out_names=tuple(out_names),
            lowering_input_output_aliases=(),
            sim_require_finite=True,
            sim_require_nnan=True,
            nc=nc,
        )
        return tuple(outs)

    devs = jax.devices()[:NCORES]
    assert len(devs) == NCORES
    mesh = Mesh(np.asarray(devs), ("core",))
    sh = NamedSharding(mesh, PartitionSpec("core"))
    in_specs = (PartitionSpec("core"),) * (n_params + n_outs)
    out_specs = (PartitionSpec("core"),) * n_outs
    donate = tuple(range(n_params, n_params + n_outs))
    sharded = jax.jit(
        shard_map(
            _body, mesh=mesh, in_specs=in_specs, out_specs=out_specs,
            check_rep=False,
        ),
        donate_argnums=donate,
        keep_unused=True,
    )
    d = {
        "fn": sharded,
        "in_names": in_names,
        "out_names": out_names,
        "zero_shapes": zero_shapes,
        "devs": devs,
        "sh": sh,
        "jax": jax,
        "make_global": jax.make_array_from_single_device_arrays,
    }
    _DISPATCH_CACHE["d"] = d
    return d


def _stage_inputs(d, in_maps):
    """Upload unique bytes through the tunnel; replicate shared tensors d2d."""
    jax = d["jax"]
    devs, sh = d["devs"], d["sh"]
    dev_in = []
    for name in d["in_names"]:
        per_core = [np.asarray(m[name]) for m in in_maps]
        rep = _REPLICATED.get(name, 1)
        if rep == 1:
            dev_in.append(jax.device_put(np.concatenate(per_core, axis=0), sh))
        else:
            shards = [None] * NCORES
            for g0 in range(0, NCORES, rep):
                src = jax.device_put(per_core[g0], devs[g0])
                shards[g0] = src
                for j in range(g0 + 1, g0 + rep):
                    shards[j] = jax.device_put(src, devs[j])
            gshape = (NCORES * per_core[0].shape[0],) + per_core[0].shape[1:]
            dev_in.append(d["make_global"](gshape, sh, shards))
    jax.block_until_ready(dev_in)
    return dev_in


def _stage_zeros(d):
    jax = d["jax"]
    return [
        jax.device_put(np.zeros((NCORES * sp[0], *sp[1:]), np_dt), d["sh"])
        for sp, np_dt in d["zero_shapes"]
    ]


def _exec_fetch(d, dev_in, zeros):
    """One dispatch round-trip: execute the NEFF on all 8 cores, fetch the
    outputs to host numpy.  `zeros` are donated output buffers."""
    out = d["fn"](*dev_in, *zeros)
    return [np.asarray(o) for o in out]


LAST_RESULT = None
LAST_EXEC_NS = None


def kernel(
    query: np.ndarray,
    support: np.ndarray,
    _trace: bool = False,
    _time_iters: int = 0,
):
    global LAST_EXEC_NS
    query = np.asarray(query, dtype=np.float32)
    support = np.asarray(support, dtype=np.float32)
    assert query.shape == (B, M, C) and support.shape == (B, N, C)

    d = _get_dispatch()
    in_maps = _prep_in_maps(query, support)
    dev_in = _stage_inputs(d, in_maps)

    # Warm-up: first execution after device idle has produced corrupted PSUM
    # accumulations; return the second execution's results.
    zsets = [_stage_zeros(d) for _ in range(2 + _time_iters)]
    d["jax"].block_until_ready(zsets)
    _exec_fetch(d, dev_in, zsets[0])
    outs = _exec_fetch(d, dev_in, zsets[1])

    if _time_iters:
        best = None
        for i in range(_time_iters):
            t0 = time.perf_counter()
            _exec_fetch(d, dev_in, zsets[2 + i])
            dt_ns = (time.perf_counter() - t0) * 1e9
            best = dt_ns if best is None else min(best, dt_ns)
        LAST_EXEC_NS = int(best)

    out = outs[d["out_names"].index("out")].reshape(NCORES, M_CORE, 2 * K)
    rows = M // (NCORES // B)
    vals = np.empty((B, M, K), dtype=np.float32)
    idx = np.empty((B, M, K), dtype=np.int32)
    for core in range(NCORES):
        b = core // (NCORES // B)
        r0 = (core % (NCORES // B)) * rows
        vals[b, r0 : r0 + rows] = out[core, :, 0:K]
        idx[b, r0 : r0 + rows] = np.rint(out[core, :, K : 2 * K]).astype(np.int32)
    return vals, idx
